# revision 1
# baseline (speedup 1.0000x reference)
"""Block-diagonal compress kernel: out = blockdiag(A) @ W @ blockdiag(B).

Shapes (full): W [8192, 8192] f32, A_blocks [128, 64, 64], B_blocks [128, 64, 64].
Sharding: row-shard W / A over 8 cores (1024 rows = 16 A-blocks each);
B replicated.  Each core computes outT = (A_bd @ W_shard @ B_bd)^T and the
host transposes each shard back on gather.

Per-core dataflow (all sizes per core):
  step 1:  T = (A_bd @ W)^T computed 128-column-chunk-wise with W as the
           matmul *stationary* operand:  matmul(lhsT=W[128 rows, 128 cols],
           rhs=blockdiag(A_even^T, A_odd^T)) -> psum [128 cols, 128 rows].
           This absorbs the transpose that a chained matmul otherwise needs.
  step 2:  outT[chunk] = matmul(lhsT=blockdiag(B_j0, B_j1), rhs=T chunk)
           at float32r (full-rate fp32 for moving free dim >= 256).

DMA layout: W is host-retiled to [G, R, 128, 1024] so each W load is one
fully contiguous 512 KB HBM read. W loads ride the SP HWDGE queue, outT
stores the Activation HWDGE queue, small preloads the gpsimd SWDGE queue —
three independent hardware DMA queues.
"""

import bass_rust
import numpy as np

import concourse.bass as bass
import concourse.mybir as mybir
from concourse.bass_utils import run_bass_kernel_spmd
from concourse.tile import TileContext

F32 = mybir.dt.float32
F32R = mybir.dt.float32r
BF16 = mybir.dt.bfloat16

N_CORES = 8
D = 8192
BLK = 64
ROWS_PC = D // N_CORES  # 1024 rows of W / out per core

_HOIST_OPCODES = {"Matmult", "DMACopy", "TensorCopy", "Memset", "Activation", "Drain"}


def _hoist_excess_matmul_waits(nc: bass.Bass, max_waits: int = 1) -> None:
    """walrus's codegen for several instruction structs (fused-LDWEIGHTS
    matmul, DMA_DIRECT2D, ...) has few sync-wait slots ("Too many sync wait
    commands"). Move excess semaphore waits off such instructions into
    standalone EventSemaphore instructions right before them on the same
    engine queue — the sequencer executes those in order, so the instruction
    still starts only after all waits pass."""
    ctr = 0
    for fnc in nc.m.functions:
        for bb in fnc.blocks:
            new = []
            for ins in bb.instructions:
                si = ins.sync_info if ins.opcode in _HOIST_OPCODES else None
                if si is not None and len(si.on_wait) > max_waits:
                    waits = list(si.on_wait)
                    for w in waits[:-max_waits]:
                        evs = mybir.InstEventSemaphore(
                            name=f"mmwaithoist-{ctr}", ins=[], outs=[]
                        )
                        ctr += 1
                        evs.engine = ins.engine
                        evs.sync_info = bass_rust.SyncInfo(on_wait=[w], on_update=[])
                        new.append(evs)
                    ins.sync_info.on_wait = waits[-max_waits:]
                new.append(ins)
            bb.instructions[:] = new


def build_nc(rows_pc: int = ROWS_PC, d: int = D, hoist: bool = True) -> bass.Bass:
    """One-core SPMD program. rows_pc/d scaled down only for sim tests.
    hoist=False keeps waits on the original instructions (CoreSim's race
    detector wants every instruction to carry its own updates; the hoisted
    variant is for walrus, whose ISA structs have too few wait slots)."""
    R = rows_pc // 128  # 128-row slabs per core (= A-block pairs)
    G = d // 1024       # column groups of 8x128
    n2 = (rows_pc + 511) // 512

    nc = bass.Bass()
    # W and atbd ship as bf16 hi/lo pairs: A@W = Ah@Wh + Al@Wh + Ah@Wl exactly
    # to ~2^-16 (bf16 products are exact in the fp32 PSUM accumulator), and
    # bf16 matmuls stream at 1 cycle/row vs fp32's 4.
    whl_ext = nc.declare_dram_parameter(
        "whl", [G, R, 2, 128, 1024], BF16, isOutput=False
    )
    ah_ext = nc.declare_dram_parameter("ah", [128, R * 128], BF16, isOutput=False)
    al_ext = nc.declare_dram_parameter("al", [128, R * 128], BF16, isOutput=False)
    bp_ext = nc.declare_dram_parameter("bpack", [128, d], F32R, isOutput=False)
    ot_ext = nc.declare_dram_parameter("outt", [d, rows_pc], F32, isOutput=True)

    with TileContext(nc) as tc:
        with (
            tc.tile_pool(name="const", bufs=1) as cpool,
            tc.tile_pool(name="wp", bufs=6) as wpool,
            tc.tile_pool(name="tg", bufs=3) as tpool,
            tc.tile_pool(name="op", bufs=4) as opool,
            tc.tile_pool(name="p1", bufs=2, space="PSUM") as p1pool,
            tc.tile_pool(name="p2", bufs=2, space="PSUM") as p2pool,
        ):
            # A hi/lo first on the scalar HWDGE queue (fast; they gate the
            # first matmul), then bpack — the queue is otherwise idle until
            # the first outT store ~25us in.
            ah = cpool.tile([128, R * 128], BF16)
            nc.scalar.dma_start(out=ah[:], in_=ah_ext[:])
            al = cpool.tile([128, R * 128], BF16)
            nc.scalar.dma_start(out=al[:], in_=al_ext[:])
            bpack_r = cpool.tile([128, d], F32R)
            nc.scalar.dma_start(out=bpack_r[:], in_=bp_ext[:])

            for g in range(G):
                # T for this column group: col = cc*rows_pc + r*128 + n
                # holds AW^T[g*1024 + cc*128 + :, :] for the core's rows.
                tg = tpool.tile([128, 8 * rows_pc], F32R)
                tgv = tg[:].rearrange("p (cc r n) -> p cc r n", cc=8, r=R)
                for r in range(R):
                    wt = wpool.tile([128, 2048], BF16)
                    nc.sync.dma_start(
                        out=wt[:].rearrange("p (h c) -> p h c", h=2),
                        in_=whl_ext[g, r].transpose([1, 0, 2]),
                    )
                    p1 = p1pool.tile([128, 1024], F32)
                    for cc in range(8):
                        cs = slice(cc * 128, (cc + 1) * 128)
                        hs = slice(cc * 128, (cc + 1) * 128)
                        ls = slice(1024 + cc * 128, 1024 + (cc + 1) * 128)
                        rs = slice(r * 128, (r + 1) * 128)
                        nc.tensor.matmul(
                            p1[:, cs], lhsT=wt[:, hs], rhs=ah[:, rs],
                            start=True, stop=False,
                        )
                        nc.tensor.matmul(
                            p1[:, cs], lhsT=wt[:, hs], rhs=al[:, rs],
                            start=False, stop=False,
                        )
                        nc.tensor.matmul(
                            p1[:, cs], lhsT=wt[:, ls], rhs=ah[:, rs],
                            start=False, stop=True,
                        )
                    nc.vector.tensor_copy(
                        tgv[:, :, r, :],
                        p1[:].rearrange("p (cc n) -> p cc n", cc=8),
                    )
                for cc in range(8):
                    j2 = 8 * g + cc
                    p2 = p2pool.tile([128, rows_pc], F32)
                    lb = bpack_r[:, j2 * 128 : (j2 + 1) * 128]
                    for s in range(n2):
                        w0 = s * 512
                        w1 = min(rows_pc, w0 + 512)
                        ts = slice(cc * rows_pc + w0, cc * rows_pc + w1)
                        nc.tensor.matmul(
                            p2[:, w0:w1], lhsT=lb, rhs=tg[:, ts],
                            start=True, stop=True,
                        )
                    ot = opool.tile([128, rows_pc], F32)
                    # split PSUM->SBUF copies between DVE and ACT (ACT is
                    # ~2x slower, so give it the smaller share)
                    if cc % 8 < 3:
                        nc.vector.tensor_copy(ot[:], p2[:])
                    else:
                        nc.scalar.copy(ot[:], p2[:])
                    nc.scalar.dma_start(
                        out=ot_ext[j2 * 128 : (j2 + 1) * 128, :], in_=ot[:]
                    )
    if hoist:
        _hoist_excess_matmul_waits(nc)
    return nc


def pack_at(a_blocks: np.ndarray) -> np.ndarray:
    """[2R, 64, 64] A blocks -> [128, R*128] with
    out[64*b + k, 128*r + 64*b + n] = A[2r+b][n, k] (transposed, pair-blockdiag)."""
    nb = a_blocks.shape[0]
    R = nb // 2
    out = np.zeros((128, R * 128), np.float32)
    at = a_blocks.transpose(0, 2, 1)
    out[0:64].reshape(64, R, 2, 64)[:, :, 0, :] = at[0::2].transpose(1, 0, 2)
    out[64:128].reshape(64, R, 2, 64)[:, :, 1, :] = at[1::2].transpose(1, 0, 2)
    return out


def pack_b(b_blocks: np.ndarray) -> np.ndarray:
    """[2J, 64, 64] B blocks -> [128, J*128] with
    out[64*b + k, 128*j + 64*b + n] = B[2j+b][k, n] (pair-blockdiag, untransposed)."""
    nb = b_blocks.shape[0]
    J = nb // 2
    out = np.zeros((128, J * 128), np.float32)
    out[0:64].reshape(64, J, 2, 64)[:, :, 0, :] = b_blocks[0::2].transpose(1, 0, 2)
    out[64:128].reshape(64, J, 2, 64)[:, :, 1, :] = b_blocks[1::2].transpose(1, 0, 2)
    return out


def pack_w(w_shard: np.ndarray):
    """[rows_pc, d] -> bf16 [G, R, 2, 128, 1024] ([...,0,:,:]=hi, [...,1,:,:]=lo)
    so each (g, r) W hi/lo pair is one contiguous 512 KB block in DRAM."""
    import ml_dtypes

    rows_pc, d = w_shard.shape
    R, G = rows_pc // 128, d // 1024
    wt = w_shard.reshape(R, 128, G, 1024).transpose(2, 0, 1, 3)
    whl = np.empty((G, R, 2, 128, 1024), ml_dtypes.bfloat16)
    whl[:, :, 0] = wt.astype(ml_dtypes.bfloat16)
    whl[:, :, 1] = (wt - whl[:, :, 0].astype(np.float32)).astype(ml_dtypes.bfloat16)
    return whl


def split_bf16(x: np.ndarray):
    import ml_dtypes

    hi = x.astype(ml_dtypes.bfloat16)
    lo = (x - hi.astype(np.float32)).astype(ml_dtypes.bfloat16)
    return hi, lo


_NC_CACHE: dict = {}


def run(W, A_blocks, B_blocks, trace: bool = False, trace_cores=None):
    W = np.asarray(W, dtype=np.float32)
    A_blocks = np.asarray(A_blocks, dtype=np.float32)
    B_blocks = np.asarray(B_blocks, dtype=np.float32)
    assert W.shape == (D, D) and A_blocks.shape == (D // BLK, BLK, BLK)

    if "nc" not in _NC_CACHE:
        _NC_CACHE["nc"] = build_nc()
    nc = _NC_CACHE["nc"]

    bp = pack_b(B_blocks)
    in_maps = []
    for c in range(N_CORES):
        whl = pack_w(W[ROWS_PC * c : ROWS_PC * (c + 1)])
        ah, al = split_bf16(pack_at(A_blocks[16 * c : 16 * (c + 1)]))
        in_maps.append(
            {"whl": whl, "ah": ah, "al": al, "bpack": bp}
        )
    res = run_bass_kernel_spmd(nc, in_maps, core_ids=list(range(N_CORES)), trace=trace, trace_cores=trace_cores)
    out = np.empty((D, D), np.float32)
    for c in range(N_CORES):
        out[ROWS_PC * c : ROWS_PC * (c + 1), :] = res.results[c]["outt"].T
    return out, res


def kernel(W, A_blocks, B_blocks):
    out, _ = run(W, A_blocks, B_blocks, trace=False)
    return out



# revision 2
# speedup vs baseline: 1.7356x; 1.7356x over previous
"""Block-diagonal compress kernel: out = blockdiag(A) @ W @ blockdiag(B).

Shapes (full): W [8192, 8192] f32, A_blocks [128, 64, 64], B_blocks [128, 64, 64].
Sharding: row-shard W / A over 8 cores (1024 rows = 16 A-blocks each);
B replicated.  Each core computes outT = (A_bd @ W_shard @ B_bd)^T and the
host transposes each shard back on gather.

The rel-err gate is 2e-2; bf16 rounding of W/A/B/T/out costs ~2.5e-3 total,
so everything ships and computes in bf16.  That halves HBM traffic vs an
f32-precision scheme: 16 MB W in + 16 MB outT back per core = 32 MB at
~360 GB/s/core ≈ 90 us DMA floor.

Per-core dataflow (all sizes per core):
  step 1:  T = (A_bd @ W)^T computed 128-column-chunk-wise with W as the
           matmul *stationary* operand:  matmul(lhsT=W[128 rows, 128 cols],
           rhs=blockdiag(A_even^T, A_odd^T)) -> psum [128 cols, 128 rows].
           This absorbs the transpose that a chained matmul otherwise needs.
  step 2:  outT[chunk] = matmul(lhsT=blockdiag(B_j0, B_j1), rhs=T chunk).

Loop structure: 2048-column supergroups (g2 in 4).  For each g2, step 1
fills a bf16 T tile [128, 16384] over 8 row slabs, then step 2 drains it
into 2 output stores of 8 column chunks each.  PSUM->SBUF copies alternate
DVE / ACT (gpsimd has no PSUM port).

DMA layout: W is host-retiled to [4, 8, 128, 2048] bf16 so each W load is
one contiguous 512 KB read with 4 KB per-partition descriptors.  outT is
stored pair-interleaved as [32, 128, 2048] bf16 for 4 KB descriptors too.
W loads ride the SP HWDGE queue; outT stores + preloads the ACT queue.
"""

import bass_rust
import numpy as np

import concourse.bass as bass
import concourse.mybir as mybir
from concourse.bass_utils import run_bass_kernel_spmd
from concourse.tile import TileContext

F32 = mybir.dt.float32
BF16 = mybir.dt.bfloat16

N_CORES = 8
D = 8192
BLK = 64
ROWS_PC = D // N_CORES  # 1024 rows of W / out per core

_HOIST_OPCODES = {"Matmult", "DMACopy", "TensorCopy", "Memset", "Activation", "Drain"}


def _hoist_excess_matmul_waits(nc: bass.Bass, max_waits: int = 1) -> None:
    """walrus's codegen for several instruction structs (fused-LDWEIGHTS
    matmul, DMA_DIRECT2D, ...) has few sync-wait slots ("Too many sync wait
    commands"). Move excess semaphore waits off such instructions into
    standalone EventSemaphore instructions right before them on the same
    engine queue — the sequencer executes those in order, so the instruction
    still starts only after all waits pass."""
    ctr = 0
    for fnc in nc.m.functions:
        for bb in fnc.blocks:
            new = []
            for ins in bb.instructions:
                si = ins.sync_info if ins.opcode in _HOIST_OPCODES else None
                if si is not None and len(si.on_wait) > max_waits:
                    waits = list(si.on_wait)
                    for w in waits[:-max_waits]:
                        evs = mybir.InstEventSemaphore(
                            name=f"mmwaithoist-{ctr}", ins=[], outs=[]
                        )
                        ctr += 1
                        evs.engine = ins.engine
                        evs.sync_info = bass_rust.SyncInfo(on_wait=[w], on_update=[])
                        new.append(evs)
                    ins.sync_info.on_wait = waits[-max_waits:]
                new.append(ins)
            bb.instructions[:] = new
    return


def build_nc(rows_pc: int = ROWS_PC, d: int = D, hoist: bool = True) -> bass.Bass:
    """One-core SPMD program."""
    R = rows_pc // 128  # 8 row slabs per core (= A-block pairs)
    G2 = d // 2048      # 4 column supergroups
    NQ = d // 256       # 32 output chunk-pairs

    nc = bass.Bass()
    wb_ext = nc.declare_dram_parameter("wb", [G2, R, 128, 2048], BF16, isOutput=False)
    ah_ext = nc.declare_dram_parameter("ah", [128, R * 128], BF16, isOutput=False)
    bp_ext = nc.declare_dram_parameter("bpack", [128, d], BF16, isOutput=False)
    ot_ext = nc.declare_dram_parameter("outt", [NQ, 128, 2048], BF16, isOutput=True)

    with TileContext(nc) as tc:
        with (
            tc.tile_pool(name="const", bufs=1) as cpool,
            tc.tile_pool(name="wp", bufs=6) as wpool,
            tc.tile_pool(name="tg", bufs=2) as tpool,
            tc.tile_pool(name="op", bufs=2) as opool,
            tc.tile_pool(name="p1", bufs=2, space="PSUM") as p1pool,
            tc.tile_pool(name="p2", bufs=2, space="PSUM") as p2pool,
        ):
            # Preloads ride the ACT queue, which is otherwise idle until the
            # first outT store.
            ah = cpool.tile([128, R * 128], BF16)
            nc.scalar.dma_start(out=ah[:], in_=ah_ext[:])
            bpack = cpool.tile([128, d], BF16)
            nc.scalar.dma_start(out=bpack[:], in_=bp_ext[:])

            cp = 0  # round-robin DVE/ACT for PSUM->SBUF copies
            for g2 in range(G2):
                # T for this supergroup: bf16 [128, h(2) cc(8) r(8) n(128)];
                # tg[c, h, cc, r, n] = AW^T[g2*2048 + h*1024 + cc*128 + c,
                #                           r*128 + n] for the core's rows.
                tg = tpool.tile([128, 2 * 8 * R * 128], BF16)
                tgv = tg[:].rearrange("p (h cc r n) -> p h cc r n", h=2, cc=8, r=R)
                for r in range(R):
                    wt = wpool.tile([128, 2048], BF16)
                    nc.sync.dma_start(out=wt[:], in_=wb_ext[g2, r])
                    for h in range(2):
                        p1 = p1pool.tile([128, 1024], F32)
                        for cc in range(8):
                            cs = slice(cc * 128, (cc + 1) * 128)
                            ws = slice(h * 1024 + cc * 128, h * 1024 + (cc + 1) * 128)
                            rs = slice(r * 128, (r + 1) * 128)
                            nc.tensor.matmul(
                                p1[:, cs], lhsT=wt[:, ws], rhs=ah[:, rs],
                                start=True, stop=True,
                            )
                        src = p1[:].rearrange("p (cc n) -> p cc n", cc=8)
                        if cp % 2 == 0:
                            nc.vector.tensor_copy(tgv[:, h, :, r, :], src)
                        else:
                            nc.scalar.copy(tgv[:, h, :, r, :], src)
                        cp += 1
                for h in range(2):
                    g = 2 * g2 + h
                    ot = opool.tile([128, 8 * rows_pc], BF16)
                    for cc in range(8):
                        j2 = 8 * g + cc
                        p2 = p2pool.tile([128, rows_pc], F32)
                        lb = bpack[:, j2 * 128 : (j2 + 1) * 128]
                        for s in range(2):
                            w0, w1 = s * 512, (s + 1) * 512
                            ts = slice((h * 8 + cc) * rows_pc + w0,
                                       (h * 8 + cc) * rows_pc + w1)
                            nc.tensor.matmul(
                                p2[:, w0:w1], lhsT=lb, rhs=tg[:, ts],
                                start=True, stop=True,
                            )
                        if cp % 2 == 0:
                            nc.vector.tensor_copy(
                                ot[:, cc * rows_pc : (cc + 1) * rows_pc], p2[:]
                            )
                        else:
                            nc.scalar.copy(
                                ot[:, cc * rows_pc : (cc + 1) * rows_pc], p2[:]
                            )
                        cp += 1
                    # 8 chunks j2 in [8g, 8g+8) = DRAM rows q in [4g, 4g+4)
                    nc.scalar.dma_start(
                        out=ot_ext[4 * g : 4 * (g + 1)].transpose([1, 0, 2]),
                        in_=ot[:].rearrange("p (q n) -> p q n", q=4),
                    )
    if hoist:
        _hoist_excess_matmul_waits(nc)
    return nc


def pack_at(a_blocks: np.ndarray) -> np.ndarray:
    """[2R, 64, 64] A blocks -> [128, R*128] with
    out[64*b + k, 128*r + 64*b + n] = A[2r+b][n, k] (transposed, pair-blockdiag)."""
    nb = a_blocks.shape[0]
    R = nb // 2
    out = np.zeros((128, R * 128), np.float32)
    at = a_blocks.transpose(0, 2, 1)
    out[0:64].reshape(64, R, 2, 64)[:, :, 0, :] = at[0::2].transpose(1, 0, 2)
    out[64:128].reshape(64, R, 2, 64)[:, :, 1, :] = at[1::2].transpose(1, 0, 2)
    return out


def pack_b(b_blocks: np.ndarray) -> np.ndarray:
    """[2J, 64, 64] B blocks -> [128, J*128] with
    out[64*b + k, 128*j + 64*b + n] = B[2j+b][k, n] (pair-blockdiag, untransposed)."""
    nb = b_blocks.shape[0]
    J = nb // 2
    out = np.zeros((128, J * 128), np.float32)
    out[0:64].reshape(64, J, 2, 64)[:, :, 0, :] = b_blocks[0::2].transpose(1, 0, 2)
    out[64:128].reshape(64, J, 2, 64)[:, :, 1, :] = b_blocks[1::2].transpose(1, 0, 2)
    return out


def pack_w(w_shard: np.ndarray):
    """[rows_pc, d] f32 -> bf16 [G2, R, 128, 2048] so each (g2, r) W tile is
    one contiguous 512 KB block in DRAM with 4 KB per-partition lines."""
    import ml_dtypes

    rows_pc, d = w_shard.shape
    R, G2 = rows_pc // 128, d // 2048
    return np.ascontiguousarray(
        w_shard.reshape(R, 128, G2, 2048).transpose(2, 0, 1, 3)
    ).astype(ml_dtypes.bfloat16)


def unpack_out(ot: np.ndarray) -> np.ndarray:
    """[NQ, 128, 2048] bf16 pair-interleaved outT -> [rows_pc, d] f32."""
    nq = ot.shape[0]
    outt = (
        np.asarray(ot, dtype=np.float32)
        .reshape(nq, 128, 2, 1024)
        .transpose(0, 2, 1, 3)
        .reshape(nq * 256, 1024)
    )
    return outt.T


_NC_CACHE: dict = {}


def run(W, A_blocks, B_blocks, trace: bool = False, trace_cores=None):
    import ml_dtypes

    W = np.asarray(W, dtype=np.float32)
    A_blocks = np.asarray(A_blocks, dtype=np.float32)
    B_blocks = np.asarray(B_blocks, dtype=np.float32)
    assert W.shape == (D, D) and A_blocks.shape == (D // BLK, BLK, BLK)

    if "nc" not in _NC_CACHE:
        _NC_CACHE["nc"] = build_nc()
    nc = _NC_CACHE["nc"]

    bp = pack_b(B_blocks).astype(ml_dtypes.bfloat16)
    in_maps = []
    for c in range(N_CORES):
        wb = pack_w(W[ROWS_PC * c : ROWS_PC * (c + 1)])
        ah = pack_at(A_blocks[16 * c : 16 * (c + 1)]).astype(ml_dtypes.bfloat16)
        in_maps.append({"wb": wb, "ah": ah, "bpack": bp})
    res = run_bass_kernel_spmd(
        nc, in_maps, core_ids=list(range(N_CORES)), trace=trace, trace_cores=trace_cores
    )
    out = np.empty((D, D), np.float32)
    for c in range(N_CORES):
        out[ROWS_PC * c : ROWS_PC * (c + 1), :] = unpack_out(res.results[c]["outt"])
    return out, res


def kernel(W, A_blocks, B_blocks):
    out, _ = run(W, A_blocks, B_blocks, trace=False)
    return out
